# revision 11
# baseline (speedup 1.0000x reference)
"""Trainium2 Bass kernel for nn_DualOutputMoE.

Math: the reference collapses the whole MoE into a single [1,1,H] vector:
    acc = sum_e combine[:,e] @ (gelu(x @ W1[e] + b1[e]) @ W2[e] + b2[e])
    out = acc / total_weight
Since combine is applied *linearly* after the gelu, we contract it with the
gelu activations first:
    u_e  = combine[:,e] @ gelu(x @ W1[e] + b1[e])          # [F]
    acc  = sum_e (u_e @ W2[e] + combine[:,e].sum() * b2[e])
which turns the second [T,F]@[F,H] matmul into an [F]@[F,H] matvec.
Only tokens actually routed to expert e (combine[:,e] != 0) contribute, so we
gather those tokens on the host (top-2 of 16 experts -> ~T/8 tokens/expert)
and the device computes a dense [C,H]@[H,F] per expert with C = padded
capacity.

Sharding: expert-parallel, 2 experts per core across 8 cores (E=16). Each
core returns its partial [1,H] accumulator; the host sums them.

Device kernel (per core, SPMD over 8 cores, bf16 with fp32 PSUM):
  mm1:  psA[tok128, f512] += xgT[h128, tok128].T @ W1[h128, f512]   (8 k-tiles)
  gelu: G = gelu(psA)                     (ACT, psum->sbuf bf16)
  cmm:  psB[1, f512] += c[tok128, 1].T @ G                          (NT tok-tiles)
  u     = psB chunks -> [1, F] -> (DRAM bounce) -> uT [128, F/128]
  mm2:  psC[1, h512] += uT[f128, 1].T @ W2[f128, h512]   (32 f-tiles, 2 experts)
"""

import sys
import math

if "/opt/trn_rl_repo" not in sys.path:
    sys.path.insert(0, "/opt/trn_rl_repo")

import numpy as np
import ml_dtypes

import concourse.bass as bass
import concourse.tile as tile
from concourse import bacc, mybir
from concourse.bass_utils import run_bass_kernel_spmd

BF16 = ml_dtypes.bfloat16
N_CORES = 8
E = 16
EPC = E // N_CORES  # experts per core
H = 1024
F = 4096
TOP_K = 2
KH = H // 128  # 8 k-tiles along H
FT = F // 128  # 32 f-tiles along F
FC = F // 512  # 8 f-chunks of 512
HC = H // 512  # 2 h-chunks of 512

_compiled_cache = {}


def _build(nt: int, has_b1: bool, reps: int = 1):
    """Build + compile the SPMD device program for NT token tiles per expert.

    reps > 1 wraps the whole body in a hardware For_i loop running it that
    many times (used by test.py for wall-clock timing).
    """
    key = (nt, has_b1, reps)
    if key in _compiled_cache:
        return _compiled_cache[key]

    C = nt * 128
    nc = bacc.Bacc("TRN2", target_bir_lowering=False, debug=False)
    f32 = mybir.dt.float32
    bf16 = mybir.dt.bfloat16

    xg_d = nc.dram_tensor("xg", [EPC, H, C], bf16, kind="ExternalInput").ap()
    cw_d = nc.dram_tensor("cw", [EPC, C], bf16, kind="ExternalInput").ap()
    w1_d = nc.dram_tensor("w1", [EPC, H, F], bf16, kind="ExternalInput").ap()
    w2_d = nc.dram_tensor("w2", [EPC, F, H], bf16, kind="ExternalInput").ap()
    if has_b1:
        b1_d = nc.dram_tensor("b1", [EPC, F], bf16, kind="ExternalInput").ap()
    acc_d = nc.dram_tensor("acc", [1, H], f32, kind="ExternalOutput").ap()
    u_d = nc.dram_tensor("u_scratch", [EPC, F], f32).ap()

    with tile.TileContext(nc) as tc:
        with (
            tc.tile_pool(name="xg", bufs=1) as xg_pool,
            tc.tile_pool(name="cw", bufs=1) as cw_pool,
            tc.tile_pool(name="w1", bufs=2) as w1_pool,
            tc.tile_pool(name="w2", bufs=4) as w2_pool,
            tc.tile_pool(name="g", bufs=3) as g_pool,
            tc.tile_pool(name="u", bufs=1) as u_pool,
            tc.tile_pool(name="small", bufs=1) as small_pool,
            tc.tile_pool(name="psA", bufs=3, space="PSUM") as psA_pool,
            tc.tile_pool(name="psB", bufs=2, space="PSUM") as psB_pool,
            tc.tile_pool(name="psC", bufs=1, space="PSUM") as psC_pool,
        ):
            # resident inputs: gathered tokens (transposed) + combine weights
            xg_sb = []
            cw_sb = []
            for e in range(EPC):
                xg_t = xg_pool.tile([128, KH, C], bf16, tag=f"xg{e}", name=f"xg{e}")
                nc.sync.dma_start(
                    xg_t[:], xg_d[e].rearrange("(k p) c -> p k c", p=128)
                )
                xg_sb.append(xg_t)
                cw_t = cw_pool.tile([128, nt], bf16, tag=f"cw{e}", name=f"cw{e}")
                nc.sync.dma_start(cw_t[:], cw_d[e].rearrange("(t p) -> p t", p=128))
                cw_sb.append(cw_t)
            b1_sb = []
            ones_sb = None
            if has_b1:
                ones_sb = small_pool.tile([1, 128], bf16, tag="ones", name="ones")
                nc.vector.memset(ones_sb[:], 1.0)
                for e in range(EPC):
                    b1_t = small_pool.tile(
                        [1, F], bf16, tag=f"b1{e}", name=f"b1{e}"
                    )
                    nc.sync.dma_start(b1_t[:], b1_d[e : e + 1, :])
                    b1_sb.append(b1_t)

            u_rows = [
                u_pool.tile([1, F], f32, tag=f"u{e}", name=f"u{e}")
                for e in range(EPC)
            ]

            def emit_phase1():
                # per-expert mm1 + gelu + combine-contraction
                for e in range(EPC):
                    for fc in range(FC):
                        w1_t = w1_pool.tile([128, KH, 512], bf16, name="w1t")
                        nc.sync.dma_start(
                            w1_t[:],
                            w1_d[e, :, fc * 512 : (fc + 1) * 512].rearrange(
                                "(k p) n -> p k n", p=128
                            ),
                        )
                        psB = psB_pool.tile([1, 512], f32, name="psB")
                        for tt in range(nt):
                            psA = psA_pool.tile([128, 512], f32, name="psA")
                            for kh in range(KH):
                                nc.tensor.matmul(
                                    psA[:],
                                    lhsT=xg_sb[e][:, kh, tt * 128 : (tt + 1) * 128],
                                    rhs=w1_t[:, kh, :],
                                    start=(kh == 0),
                                    stop=(kh == KH - 1) and not has_b1,
                                )
                            if has_b1:
                                nc.tensor.matmul(
                                    psA[:],
                                    lhsT=ones_sb[:],
                                    rhs=b1_sb[e][:, fc * 512 : (fc + 1) * 512],
                                    start=False,
                                    stop=True,
                                )
                            g_t = g_pool.tile([128, 512], bf16, name="gt")
                            nc.scalar.activation(
                                g_t[:], psA[:], mybir.ActivationFunctionType.Gelu
                            )
                            nc.tensor.matmul(
                                psB[:],
                                lhsT=cw_sb[e][:, tt : tt + 1],
                                rhs=g_t[:],
                                start=(tt == 0),
                                stop=(tt == nt - 1),
                            )
                        nc.vector.tensor_copy(
                            u_rows[e][:, fc * 512 : (fc + 1) * 512], psB[:]
                        )
                    # u -> DRAM (for the partition-transpose bounce)
                    nc.sync.dma_start(u_d[e : e + 1, :], u_rows[e][:])

            def emit_phase2():
                # transpose u, then mm2 accumulating both experts into psC
                uT_bf = []
                for e in range(EPC):
                    uT_f = small_pool.tile(
                        [128, FT], f32, tag=f"uTf{e}", name=f"uTf{e}"
                    )
                    nc.sync.dma_start(
                        uT_f[:], u_d[e].rearrange("(j p) -> p j", p=128)
                    )
                    uT_b = small_pool.tile(
                        [128, FT], bf16, tag=f"uTb{e}", name=f"uTb{e}"
                    )
                    nc.vector.tensor_copy(uT_b[:], uT_f[:])
                    uT_bf.append(uT_b)

                psC = [
                    psC_pool.tile([1, 512], f32, tag=f"psC{hc}", name=f"psC{hc}")
                    for hc in range(HC)
                ]
                for e in range(EPC):
                    for ft in range(FT):
                        w2_t = w2_pool.tile([128, HC, 512], bf16, name="w2t")
                        nc.sync.dma_start(
                            w2_t[:],
                            w2_d[e, ft * 128 : (ft + 1) * 128, :].rearrange(
                                "p (h n) -> p h n", h=HC
                            ),
                        )
                        for hc in range(HC):
                            nc.tensor.matmul(
                                psC[hc][:],
                                lhsT=uT_bf[e][:, ft : ft + 1],
                                rhs=w2_t[:, hc, :],
                                start=(e == 0 and ft == 0),
                                stop=(e == EPC - 1 and ft == FT - 1),
                            )

                out_sb = small_pool.tile([1, H], f32, tag="out", name="out")
                for hc in range(HC):
                    nc.vector.tensor_copy(
                        out_sb[:, hc * 512 : (hc + 1) * 512], psC[hc][:]
                    )
                nc.sync.dma_start(acc_d[:], out_sb[:])

            if reps > 1:
                with tc.For_i(0, reps, 1):
                    emit_phase1()
                    emit_phase2()
            else:
                emit_phase1()
                emit_phase2()

    nc.compile()
    _compiled_cache[key] = nc
    return nc


def _prep_inputs(input_tensor, Wg, bg, W1, b1, W2, b2):
    """Host-side gating, top-k, gather, bf16 conversion. Returns
    (in_maps, nt, has_b1, csum, total_weight)."""
    B, S, _ = input_tensor.shape
    T = B * S
    x = np.ascontiguousarray(input_tensor.reshape(T, H)).astype(np.float32)

    scores = x @ Wg.astype(np.float32) + bg.astype(np.float32)
    order = np.argsort(-scores, axis=1, kind="stable")
    top_i = order[:, :TOP_K]
    top_v = np.take_along_axis(scores, top_i, axis=1).astype(np.float64)
    ex = np.exp(top_v - top_v.max(axis=1, keepdims=True))
    top_w = ex / ex.sum(axis=1, keepdims=True)
    total_weight = float(top_w.sum())

    flat_e = top_i.ravel()
    flat_t = np.repeat(np.arange(T), TOP_K)
    flat_w = top_w.ravel()
    sort = np.argsort(flat_e, kind="stable")
    flat_e, flat_t, flat_w = flat_e[sort], flat_t[sort], flat_w[sort]
    counts = np.bincount(flat_e, minlength=E)
    starts = np.concatenate([[0], np.cumsum(counts)])

    nt = max(1, math.ceil(counts.max() / 128))
    C = nt * 128

    xg = np.zeros((E, H, C), dtype=BF16)
    cw = np.zeros((E, C), dtype=BF16)
    csum = np.zeros(E, dtype=np.float64)
    for e in range(E):
        lo, hi = starts[e], starts[e + 1]
        if hi > lo:
            toks = flat_t[lo:hi]
            xg[e, :, : hi - lo] = x[toks].T.astype(BF16)
            cw[e, : hi - lo] = flat_w[lo:hi].astype(BF16)
            csum[e] = flat_w[lo:hi].sum()

    w1_bf = W1.astype(BF16)
    w2_bf = W2.astype(BF16)
    has_b1 = bool(np.any(b1))

    in_maps = []
    for i in range(N_CORES):
        m = {
            "xg": xg[EPC * i : EPC * (i + 1)],
            "cw": cw[EPC * i : EPC * (i + 1)],
            "w1": w1_bf[EPC * i : EPC * (i + 1)],
            "w2": w2_bf[EPC * i : EPC * (i + 1)],
        }
        if has_b1:
            m["b1"] = b1[EPC * i : EPC * (i + 1)].astype(BF16)
        in_maps.append(m)
    return in_maps, nt, has_b1, csum, total_weight


def _finalize(results, csum, b2, total_weight):
    acc = np.zeros(H, dtype=np.float64)
    for i in range(N_CORES):
        acc += results[i]["acc"].reshape(H).astype(np.float64)
    acc += csum @ b2.astype(np.float64)
    return (acc / total_weight).reshape(1, 1, H).astype(np.float32)


def kernel(input_tensor, Wg, bg, W1, b1, W2, b2):
    in_maps, nt, has_b1, csum, total_weight = _prep_inputs(
        input_tensor, Wg, bg, W1, b1, W2, b2
    )
    nc = _build(nt, has_b1)
    global _last_in_maps
    _last_in_maps = in_maps
    res = run_bass_kernel_spmd(nc, in_maps, core_ids=list(range(N_CORES)))
    return _finalize(res.results, csum, b2, total_weight)


# revision 18
# speedup vs baseline: 1.1374x; 1.1374x over previous
"""Trainium2 Bass kernel for nn_DualOutputMoE.

Math: the reference collapses the whole MoE into a single [1,1,H] vector:
    acc = sum_e combine[:,e] @ (gelu(x @ W1[e] + b1[e]) @ W2[e] + b2[e])
    out = acc / total_weight
Since combine is applied *linearly* after the gelu, we contract it with the
gelu activations first:
    u_e  = combine[:,e] @ gelu(x @ W1[e] + b1[e])          # [F]
    acc  = sum_e (u_e @ W2[e] + combine[:,e].sum() * b2[e])
which turns the second [T,F]@[F,H] matmul into an [F]@[F,H] matvec.
Only tokens actually routed to expert e (combine[:,e] != 0) contribute, so we
gather those tokens on the host (top-2 of 16 experts -> ~T/8 tokens/expert)
and the device computes a dense [C,H]@[H,F] per expert with C = padded
capacity.

Sharding: expert-parallel, 2 experts per core across 8 cores (E=16). Each
core returns its partial [1,H] accumulator; the host sums them.

Device kernel (per core, SPMD over 8 cores, bf16 with fp32 PSUM):
  mm1:  psA[tok128, f512] += xgT[h128, tok128].T @ W1[h128, f512]   (8 k-tiles)
  gelu: G = gelu(psA)                     (ACT, psum->sbuf bf16)
  cmm:  psB[1, f512] += c[tok128, 1].T @ G                          (NT tok-tiles)
  u     -> [8, 512] sbuf -> DRAM -> uT [128, F/128] (partition transpose)
  mm2:  psC[1, h512] += uT[f128, 1].T @ W2[f128, h512]   (32 f-tiles, 2 experts)

Schedule: expert 0's mm2 is interleaved into expert 1's phase 1 (PE program
order is emission order), and W2 tiles are prefetched into SBUF during phase 1
so mm2 never waits on DMA.
"""

import sys
import math

if "/opt/trn_rl_repo" not in sys.path:
    sys.path.insert(0, "/opt/trn_rl_repo")

import numpy as np
import ml_dtypes

import concourse.bass as bass
import concourse.tile as tile
from concourse import bacc, mybir
from concourse.bass_utils import run_bass_kernel_spmd

BF16 = ml_dtypes.bfloat16
N_CORES = 8
E = 16
EPC = E // N_CORES  # experts per core
H = 1024
F = 4096
TOP_K = 2
KH = H // 128  # 8 k-tiles along H
FT = F // 128  # 32 f-tiles along F
FC = F // 512  # 8 f-chunks of 512
HC = H // 512  # 2 h-chunks of 512

_compiled_cache = {}


def _build(nt: int, has_b1: bool, reps: int = 1):
    """Build + compile the SPMD device program for NT token tiles per expert.

    reps > 1 wraps the whole body in a hardware For_i loop running it that
    many times (used by test.py for wall-clock timing).
    """
    key = (nt, has_b1, reps)
    if key in _compiled_cache:
        return _compiled_cache[key]

    C = nt * 128
    nc = bacc.Bacc("TRN2", target_bir_lowering=False, debug=False)
    f32 = mybir.dt.float32
    bf16 = mybir.dt.bfloat16

    xg_d = nc.dram_tensor("xg", [EPC, H, C], bf16, kind="ExternalInput").ap()
    cw_d = nc.dram_tensor("cw", [EPC, C], bf16, kind="ExternalInput").ap()
    w1_d = nc.dram_tensor("w1", [EPC, H, F], bf16, kind="ExternalInput").ap()
    w2_d = nc.dram_tensor("w2", [EPC, F, H], bf16, kind="ExternalInput").ap()
    if has_b1:
        b1_d = nc.dram_tensor("b1", [EPC, F], bf16, kind="ExternalInput").ap()
    acc_d = nc.dram_tensor("acc", [1, H], f32, kind="ExternalOutput").ap()
    u_d = nc.dram_tensor("u_scratch", [EPC, F], f32).ap()

    with tile.TileContext(nc) as tc:
        with (
            tc.tile_pool(name="xg", bufs=1) as xg_pool,
            tc.tile_pool(name="cw", bufs=1) as cw_pool,
            tc.tile_pool(name="w1", bufs=2) as w1_pool,
            tc.tile_pool(name="w2", bufs=20) as w2_pool,
            tc.tile_pool(name="g", bufs=3) as g_pool,
            tc.tile_pool(name="u", bufs=1) as u_pool,
            tc.tile_pool(name="small", bufs=1) as small_pool,
            tc.tile_pool(name="psA", bufs=3, space="PSUM") as psA_pool,
            tc.tile_pool(name="psB", bufs=2, space="PSUM") as psB_pool,
            tc.tile_pool(name="psC", bufs=1, space="PSUM") as psC_pool,
        ):
            xg_sb, cw_sb, b1_sb, u8, uT_f, uT_b = [], [], [], [], [], []
            ones_sb = None
            for e in range(EPC):
                xg_sb.append(
                    xg_pool.tile([128, KH, C], bf16, tag=f"xg{e}", name=f"xg{e}")
                )
                cw_sb.append(
                    cw_pool.tile([128, nt], bf16, tag=f"cw{e}", name=f"cw{e}")
                )
                u8.append(u_pool.tile([1, F], f32, tag=f"u{e}", name=f"u{e}"))
                uT_f.append(
                    small_pool.tile([128, FT], f32, tag=f"uTf{e}", name=f"uTf{e}")
                )
                uT_b.append(
                    small_pool.tile([128, FT], bf16, tag=f"uTb{e}", name=f"uTb{e}")
                )

            def load_xg(e, kh):
                # per-k-tile loads so the first matmul starts after ~0.3 MB
                nc.sync.dma_start(
                    xg_sb[e][:, kh, :], xg_d[e, kh * 128 : (kh + 1) * 128, :]
                )

            def load_cw(e):
                nc.sync.dma_start(
                    cw_sb[e][:], cw_d[e].rearrange("(t p) -> p t", p=128)
                )

            if has_b1:
                ones_sb = small_pool.tile([1, 128], bf16, tag="ones", name="ones")
                nc.vector.memset(ones_sb[:], 1.0)
                for e in range(EPC):
                    b1_t = small_pool.tile([1, F], bf16, tag=f"b1{e}", name=f"b1{e}")
                    nc.sync.dma_start(b1_t[:], b1_d[e : e + 1, :])
                    b1_sb.append(b1_t)

            def w2_prefetch(e, ft):
                w2_t = w2_pool.tile([128, HC, 512], bf16, name="w2t")
                nc.sync.dma_start(
                    w2_t[:],
                    w2_d[e, ft * 128 : (ft + 1) * 128, :].rearrange(
                        "p (h n) -> p h n", h=HC
                    ),
                )
                return w2_t

            # software-pipelined state (emitted later than produced)
            cmm_q = []  # queued combine-matvecs: (e, fc, tt, g_tile)
            mm2_q = []  # queued mm2 chunk batches: (e, fc, [w2 tiles])
            state = {"mm2_count": 0}
            N_MM2 = EPC * FT * HC

            def pop_cmm():
                if not cmm_q:
                    return
                e, fc, tt, g_t = cmm_q.pop(0)
                nc.tensor.matmul(
                    psB_cur[(e, fc)][:],
                    lhsT=cw_sb[e][:, tt : tt + 1],
                    rhs=g_t[:],
                    start=(tt == 0),
                    stop=(tt == nt - 1),
                )
                if tt == nt - 1:
                    finish_chunk(e, fc)

            def finish_chunk(e, fc):
                # psB -> u8 row -> DRAM -> uT columns -> bf16 cast; then queue
                # this chunk's mm2 work
                psB = psB_cur.pop((e, fc))
                nc.vector.tensor_copy(
                    u8[e][:, fc * 512 : (fc + 1) * 512], psB[:]
                )
                nc.sync.dma_start(
                    u_d[e : e + 1, fc * 512 : (fc + 1) * 512],
                    u8[e][:, fc * 512 : (fc + 1) * 512],
                )
                nc.sync.dma_start(
                    uT_f[e][:, 4 * fc : 4 * fc + 4],
                    u_d[e, fc * 512 : (fc + 1) * 512].rearrange(
                        "(j p) -> p j", p=128
                    ),
                )
                nc.vector.tensor_copy(
                    uT_b[e][:, 4 * fc : 4 * fc + 4],
                    uT_f[e][:, 4 * fc : 4 * fc + 4],
                )
                w2_tiles = [w2_prefetch(e, ft) for ft in range(4 * fc, 4 * fc + 4)]
                mm2_q.append((e, fc, w2_tiles))

            def pop_mm2():
                if not mm2_q:
                    return
                e, fc, w2_tiles = mm2_q.pop(0)
                for j, ft in enumerate(range(4 * fc, 4 * fc + 4)):
                    for hc in range(HC):
                        nc.tensor.matmul(
                            psC[hc][:],
                            lhsT=uT_b[e][:, ft : ft + 1],
                            rhs=w2_tiles[j][:, hc, :],
                            start=(state["mm2_count"] == 0),
                            stop=(state["mm2_count"] == N_MM2 - 1),
                        )
                        state["mm2_count"] += 1

            def load_w1(e, fc, split=False):
                w1_t = w1_pool.tile([128, KH, 512], bf16, name="w1t")
                if split:  # per-k-tile DMAs: first matmul waits on 128KB only
                    for kh in range(KH):
                        nc.sync.dma_start(
                            w1_t[:, kh, :],
                            w1_d[
                                e,
                                kh * 128 : (kh + 1) * 128,
                                fc * 512 : (fc + 1) * 512,
                            ],
                        )
                else:
                    nc.sync.dma_start(
                        w1_t[:],
                        w1_d[e, :, fc * 512 : (fc + 1) * 512].rearrange(
                            "(k p) n -> p k n", p=128
                        ),
                    )
                return w1_t

            def mm1_block(e, fc, w1_t=None):
                """mm1s for one f-chunk (w1 DMA + nt token-tile groups); the
                gelu-dependent cmms are emitted one tt-group later via cmm_q."""
                if w1_t is None:
                    w1_t = load_w1(e, fc)
                psB_cur[(e, fc)] = psB_pool.tile([1, 512], f32, name="psB")
                for tt in range(nt):
                    psA = psA_pool.tile([128, 512], f32, name="psA")
                    for kh in range(KH):
                        nc.tensor.matmul(
                            psA[:],
                            lhsT=xg_sb[e][:, kh, tt * 128 : (tt + 1) * 128],
                            rhs=w1_t[:, kh, :],
                            start=(kh == 0),
                            stop=(kh == KH - 1) and not has_b1,
                        )
                    if has_b1:
                        nc.tensor.matmul(
                            psA[:],
                            lhsT=ones_sb[:],
                            rhs=b1_sb[e][:, fc * 512 : (fc + 1) * 512],
                            start=False,
                            stop=True,
                        )
                    # emit the PREVIOUS group's cmm here: its gelu ran while
                    # this group's mm1s streamed, so PE never waits on ACT
                    pop_cmm()
                    g_t = g_pool.tile([128, 512], bf16, name="gt")
                    nc.scalar.activation(
                        g_t[:], psA[:], mybir.ActivationFunctionType.Gelu
                    )
                    cmm_q.append((e, fc, tt, g_t))

            def emit_body():
                psC.clear()
                psC.extend(
                    psC_pool.tile([1, 512], f32, tag=f"psC{hc}", name=f"psC{hc}")
                    for hc in range(HC)
                )
                state["mm2_count"] = 0
                # startup: only e0's k0 slice + w1 fc0 k0 block the first matmul
                load_xg(0, 0)
                load_cw(0)
                w1_first = load_w1(0, 0, split=True)
                for kh in range(1, KH):
                    load_xg(0, kh)
                for e in range(EPC):
                    for fc in range(FC):
                        if e == 0 and fc < KH:  # spread e1's input loads out
                            load_xg(1, fc)
                            if fc == 0:
                                load_cw(1)
                        mm1_block(e, fc, w1_t=w1_first if (e, fc) == (0, 0) else None)
                        pop_mm2()
                # drain the pipeline tails
                while cmm_q:
                    pop_cmm()
                while mm2_q:
                    pop_mm2()

                out_sb = small_pool.tile([1, H], f32, tag="out", name="out")
                for hc in range(HC):
                    nc.vector.tensor_copy(
                        out_sb[:, hc * 512 : (hc + 1) * 512], psC[hc][:]
                    )
                nc.sync.dma_start(acc_d[:], out_sb[:])

            psC = []
            psB_cur = {}
            if reps > 1:
                with tc.For_i(0, reps, 1):
                    emit_body()
            else:
                emit_body()

    nc.compile()
    _compiled_cache[key] = nc
    return nc


def _prep_inputs(input_tensor, Wg, bg, W1, b1, W2, b2):
    """Host-side gating, top-k, gather, bf16 conversion. Returns
    (in_maps, nt, has_b1, csum, total_weight)."""
    B, S, _ = input_tensor.shape
    T = B * S
    x = np.ascontiguousarray(input_tensor.reshape(T, H)).astype(np.float32)

    scores = x @ Wg.astype(np.float32) + bg.astype(np.float32)
    order = np.argsort(-scores, axis=1, kind="stable")
    top_i = order[:, :TOP_K]
    top_v = np.take_along_axis(scores, top_i, axis=1).astype(np.float64)
    ex = np.exp(top_v - top_v.max(axis=1, keepdims=True))
    top_w = ex / ex.sum(axis=1, keepdims=True)
    total_weight = float(top_w.sum())

    flat_e = top_i.ravel()
    flat_t = np.repeat(np.arange(T), TOP_K)
    flat_w = top_w.ravel()
    sort = np.argsort(flat_e, kind="stable")
    flat_e, flat_t, flat_w = flat_e[sort], flat_t[sort], flat_w[sort]
    counts = np.bincount(flat_e, minlength=E)
    starts = np.concatenate([[0], np.cumsum(counts)])

    nt = max(1, math.ceil(counts.max() / 128))
    C = nt * 128

    xg = np.zeros((E, H, C), dtype=BF16)
    cw = np.zeros((E, C), dtype=BF16)
    csum = np.zeros(E, dtype=np.float64)
    for e in range(E):
        lo, hi = starts[e], starts[e + 1]
        if hi > lo:
            toks = flat_t[lo:hi]
            xg[e, :, : hi - lo] = x[toks].T.astype(BF16)
            cw[e, : hi - lo] = flat_w[lo:hi].astype(BF16)
            csum[e] = flat_w[lo:hi].sum()

    w1_bf = W1.astype(BF16)
    w2_bf = W2.astype(BF16)
    has_b1 = bool(np.any(b1))

    in_maps = []
    for i in range(N_CORES):
        m = {
            "xg": xg[EPC * i : EPC * (i + 1)],
            "cw": cw[EPC * i : EPC * (i + 1)],
            "w1": w1_bf[EPC * i : EPC * (i + 1)],
            "w2": w2_bf[EPC * i : EPC * (i + 1)],
        }
        if has_b1:
            m["b1"] = b1[EPC * i : EPC * (i + 1)].astype(BF16)
        in_maps.append(m)
    return in_maps, nt, has_b1, csum, total_weight


def _finalize(results, csum, b2, total_weight):
    acc = np.zeros(H, dtype=np.float64)
    for i in range(N_CORES):
        acc += results[i]["acc"].reshape(H).astype(np.float64)
    acc += csum @ b2.astype(np.float64)
    return (acc / total_weight).reshape(1, 1, H).astype(np.float32)


def kernel(input_tensor, Wg, bg, W1, b1, W2, b2):
    in_maps, nt, has_b1, csum, total_weight = _prep_inputs(
        input_tensor, Wg, bg, W1, b1, W2, b2
    )
    nc = _build(nt, has_b1)
    global _last_in_maps
    _last_in_maps = in_maps
    res = run_bass_kernel_spmd(nc, in_maps, core_ids=list(range(N_CORES)))
    return _finalize(res.results, csum, b2, total_weight)
